# revision 20
# baseline (speedup 1.0000x reference)
"""Multi-head attention + residual + LayerNorm on 8 Trainium2 cores.

Model (per reference):
  Q/K/V = x @ W.T  (torch Linear), 16 heads x d_k=64, softmax(QK^T/8) @ V,
  out-proj, residual with query, LayerNorm.

Sharding: core c = (batch b = c//4, head-group hg = c%4 of 4 heads).

Launch 1 (per core): QKV projections for its 4 heads on its batch
  (fp8 DoubleRow matmuls: two 128-deep K-tiles per instruction),
  attention scores^T = K @ Q^T (k on partitions, bf16, row-packed head
  pairs) in qq-major order, exp on the Scalar engine (the kernel's
  bottleneck: one exp per score element, ACT-only), then context with the
  exp-scores tile as the STATIONARY operand:
  ctx[q,d] = sum_c sx(c)[:,q].T @ Vaug(c) -- output [q=128, 65] costs 65
  PE rows per matmul instead of 512, and the 65th column (ones appended
  to V) accumulates the softmax denominator. qq-major score order lets
  each q-block group's ctx run as soon as its quarter of the exps is
  done, so ctx/normalize/transpose overlap exp instead of trailing it.
  Context is normalized in fp32 (DVE reciprocal of psum col 64), then
  PE-transposed back to ctx^T for launch 2.
Launch 2 (per core): 512 rows of B*S=4096: out-projection of the
  already-normalized ctx^T, residual add, LayerNorm. When ln_gamma is
  all-ones and ln_beta all-zeros (the eval-mode default), a variant
  without the gamma/beta tail is used.
"""

import numpy as np
import ml_dtypes

import concourse.bass as bass
import concourse.bacc as bacc
import concourse.tile as tile
from concourse import mybir
from concourse.bass_utils import run_bass_kernel_spmd

BF16 = mybir.dt.bfloat16
F32 = mybir.dt.float32
FP8 = mybir.dt.float8e4
NPBF16 = ml_dtypes.bfloat16
NPFP8 = ml_dtypes.float8_e4m3

B, S, D = 2, 2048, 1024
H = 16
DK = 64
N_CORES = 8
H_LOC = 4          # heads per core
HP_LOC = 2         # head pairs per core
NCH = S // 128     # 16 k-chunks (also 16 q-blocks)
NIC = D // 128     # 8 contraction chunks
SLOT_MM = 3        # 512-col score matmuls per psum slot -> [128, 1536]
EPS = 1e-5
DR = mybir.MatmulPerfMode.DoubleRow

_cache = {}


def _dram3(t, part, mid, inner, off0=0, row_len=None):
    """AP over dram tensor t: [part partitions, mid blocks of 128 rows,
    inner cols], starting at element offset off0."""
    a = t.ap()
    rl = row_len if row_len is not None else t.shape[1]
    return bass.AP(
        tensor=a.tensor,
        offset=a.offset + off0,
        ap=[[rl, part], [128 * rl, mid], [1, inner]],
    )


def build_kernel1():
    nc = bacc.Bacc("TRN2", target_bir_lowering=False, debug=False)

    xq = nc.dram_tensor("xq", [D, S], FP8, kind="ExternalInput")
    xk = nc.dram_tensor("xk", [D, S], FP8, kind="ExternalInput")
    xv = nc.dram_tensor("xv", [D, S], FP8, kind="ExternalInput")
    wqkv = nc.dram_tensor("wqkv", [D, 768], FP8, kind="ExternalInput")
    ident = nc.dram_tensor("ident", [128, 128], BF16, kind="ExternalInput")
    ctxT = nc.dram_tensor("ctxT", [256, S], BF16, kind="ExternalOutput")

    with tile.TileContext(nc) as tc:
        with (
            tc.tile_pool(name="wp", bufs=1) as wp,
            tc.tile_pool(name="qk", bufs=1) as qkp,
            tc.tile_pool(name="va", bufs=1) as vap,
            tc.tile_pool(name="xp", bufs=1) as xp,
            tc.tile_pool(name="cn", bufs=1) as cnp,
            tc.tile_pool(name="sx", bufs=21) as sxp,
            tc.tile_pool(name="rc", bufs=4) as recp,
            tc.tile_pool(name="ps", bufs=2, space="PSUM") as psp,
            tc.tile_pool(name="pc", bufs=2, space="PSUM") as pcp,
        ):
            # Q^T/K^T in fp8 DoubleRow layout: two tiles of 2 heads x 32 dk
            # partitions each (SBUF AP base partition must be 0/32/64), free
            # dim = (dk-half, S); scores matmuls then run at 0.5 cycles/row
            # with the two dk-halves as the packed K-tiles.
            qta = qkp.tile([64, 2, S], FP8)
            qtb = qkp.tile([64, 2, S], FP8)
            kta = qkp.tile([64, 2, S], FP8)
            ktb = qkp.tile([64, 2, S], FP8)
            vaug = vap.tile([128, HP_LOC, NCH, 130], BF16)
            w_a = wp.tile([128, NIC, 768], FP8)   # [.., ic, q|k|v cols]
            x_q = xp.tile([128, NIC, S], FP8)
            x_k = xp.tile([128, NIC, S], FP8)
            x_v = xp.tile([128, NIC, S], FP8)
            idt = cnp.tile([128, 128], BF16)
            ctxn = cnp.tile([128, H_LOC, NCH, DK], BF16)
            ctxT_sb = cnp.tile([128, 2, S], BF16)

            # ---- input DMAs: few, large, ordered so attention starts early.
            # x tensors arrive in 512-column chunks (one DMA per chunk moves
            # all 8 ic blocks); xv interleaves early so the dripped V
            # projections never stall the in-order PE queue.
            def w_cols(c0, n):
                a = wqkv.ap()
                return bass.AP(
                    tensor=a.tensor, offset=a.offset + c0,
                    ap=[[768, 128], [128 * 768, NIC], [1, n]],
                )

            def x_chunk(dst, src, c4):
                nc.sync.dma_start(
                    out=dst[:, :, 512 * c4 : 512 * c4 + 512],
                    in_=_dram3(src, 128, NIC, 512, off0=512 * c4),
                )

            nc.sync.dma_start(out=w_a[:, :, 0:512], in_=w_cols(0, 512))
            x_chunk(x_k, xk, 0)
            x_chunk(x_q, xq, 0)
            x_chunk(x_k, xk, 1)
            x_chunk(x_k, xk, 2)
            x_chunk(x_k, xk, 3)
            x_chunk(x_q, xq, 1)
            nc.sync.dma_start(out=w_a[:, :, 512:768], in_=w_cols(512, 256))
            for c4 in range(4):
                x_chunk(x_v, xv, c4)
            x_chunk(x_q, xq, 2)
            x_chunk(x_q, xq, 3)
            nc.sync.dma_start(out=idt[:], in_=ident[:])
            nc.vector.memset(vaug[:], 1.0)

            # ---- Q^T / K^T projections: fp8 DoubleRow, 512 cols at a time.
            # w_a cols 0:256 = Q, 256:512 = K, each [dk-half, head, dk%32]
            # permuted host-side so one matmul emits all 4 heads' half.
            def proj_unit(t, s4, half):
                xt, ta, tb = ((x_q, qta, qtb), (x_k, kta, ktb))[t]
                pt = psp.tile([128, 512], F32, tag="slot", name="pt")
                for i in range(NIC // 2):
                    nc.tensor.matmul(
                        pt[:],
                        w_a[:, 2 * i : 2 * i + 2,
                            256 * t + 128 * half : 256 * t + 128 * half + 128],
                        xt[:, 2 * i : 2 * i + 2, 512 * s4 : 512 * s4 + 512],
                        start=(i == 0),
                        stop=(i == NIC // 2 - 1),
                        perf_mode=DR,
                    )
                nc.vector.tensor_copy(
                    out=ta[0:64, half, 512 * s4 : 512 * s4 + 512], in_=pt[0:64, :]
                )
                nc.vector.tensor_copy(
                    out=tb[0:64, half, 512 * s4 : 512 * s4 + 512], in_=pt[64:128, :]
                )

            def v_proj(c):
                pv = pcp.tile([128, 256], F32, tag="sm", name="pv")
                for i in range(NIC // 2):
                    nc.tensor.matmul(
                        pv[:],
                        x_v[:, 2 * i : 2 * i + 2, 128 * c : 128 * c + 128],
                        w_a[:, 2 * i : 2 * i + 2, 512:768],
                        start=(i == 0),
                        stop=(i == NIC // 2 - 1),
                        perf_mode=DR,
                    )
                for hp2 in range(HP_LOC):
                    nc.vector.tensor_copy(
                        out=vaug[:, hp2, c, 0:130].rearrange(
                            "p (two f) -> p two f", two=2
                        )[:, :, 0:64],
                        in_=pv[:, 128 * hp2 : 128 * hp2 + 128].rearrange(
                            "p (two f) -> p two f", two=2
                        ),
                    )

            # ---- scores slot/exp machinery -------------------------------
            state = {"slot": None, "fill": 0}
            pend = []
            sx_map = {}  # (h, c, qq) -> (sx_tile, pos)

            def flush_slot():
                if state["slot"] is None or state["fill"] == 0:
                    return
                sx = sxp.tile([128, SLOT_MM * 512], BF16, tag="sx")
                nc.scalar.activation(
                    out=sx[:, 0 : state["fill"] * 512],
                    in_=state["slot"][:, 0 : state["fill"] * 512],
                    func=mybir.ActivationFunctionType.Exp,
                    scale=0.125,
                )
                for key, pos in pend:
                    sx_map[key] = (sx, pos)
                pend.clear()
                state["slot"] = None
                state["fill"] = 0

            def scores_unit(h, c, qq):
                if state["slot"] is None:
                    state["slot"] = psp.tile(
                        [128, SLOT_MM * 512], F32, tag="slot", name="slot"
                    )
                    state["fill"] = 0
                ktx, qtx = (kta, qta) if h < 2 else (ktb, qtb)
                hb = 32 * (h % 2)
                nc.tensor.matmul(
                    state["slot"][:, 512 * state["fill"] : 512 * state["fill"] + 512],
                    ktx[hb : hb + 32, :, 128 * c : 128 * c + 128],
                    qtx[hb : hb + 32, :, 512 * qq : 512 * qq + 512],
                    start=True,
                    stop=True,
                    perf_mode=DR,
                )
                pend.append(((h, c, qq), state["fill"]))
                state["fill"] += 1
                if state["fill"] == SLOT_MM:
                    flush_slot()

            def ctx_chain(h, qb):
                # full chain for one q-block: ctx mms -> recip -> normalize
                # -> transpose -> copy to ctx^T staging (DMA per head/group)
                hp, hh = h // 2, h % 2
                if any(k[0] == h and k[2] == qb // 4 for k, _ in pend):
                    flush_slot()
                pctx = pcp.tile([128, 65], F32, tag="sm", name="pctx")
                for c in range(NCH):
                    sx, pos = sx_map[(h, c, qb // 4)]
                    off = 512 * pos + 128 * (qb % 4)
                    nc.tensor.matmul(
                        pctx[:],
                        sx[:, off : off + 128],
                        vaug[:, hp, c, 65 * hh : 65 * hh + 65],
                        start=(c == 0),
                        stop=(c == NCH - 1),
                    )
                rec = recp.tile([128, 1], F32, tag="rc", name="rec")
                nc.vector.reciprocal(out=rec[:], in_=pctx[:, 64:65])
                nc.vector.tensor_scalar_mul(
                    out=ctxn[:, h, qb, :], in0=pctx[:, 0:64], scalar1=rec[:]
                )
                ptr = pcp.tile([64, 128], BF16, tag="sm", name="ptr")
                nc.tensor.matmul(ptr[:], ctxn[:, h, qb, :], idt[:], is_transpose=True)
                nc.vector.tensor_copy(
                    out=ctxT_sb[64 * hh : 64 * hh + 64, hp, 128 * qb : 128 * qb + 128],
                    in_=ptr[:],
                )
                if h == H_LOC - 1:
                    if qb % 4 == 3:  # last head: ship per group to shrink tail
                        nc.sync.dma_start(
                            out=ctxT[64 * h : 64 * h + 64,
                                     512 * (qb // 4) : 512 * (qb // 4) + 512],
                            in_=ctxT_sb[64 * hh : 64 * hh + 64, hp,
                                        512 * (qb // 4) : 512 * (qb // 4) + 512],
                        )
                elif qb == NCH - 1:
                    nc.sync.dma_start(
                        out=ctxT[64 * h : 64 * h + 64, :],
                        in_=ctxT_sb[64 * hh : 64 * hh + 64, hp, :],
                    )

            # ---- emission schedule --------------------------------------
            # Per head: qq-major scores; group g's ctx chains interleave
            # with group g+1's scores. Projections are emitted on demand
            # (just before their first reader); V projection drips through
            # head 0 group 0; hp1 projections drip through h0 g>=2 / h1.
            proj_done = set()

            def ensure_proj(t, s4):
                if (t, s4) not in proj_done:
                    proj_done.add((t, s4))
                    proj_unit(t, s4, 0)
                    proj_unit(t, s4, 1)

            chains = []  # deferred ctx chains (h, qb)
            started = [False]  # chains start draining one group late
            for h in range(H_LOC):
                for g in range(4):
                    ensure_proj(0, g)
                    for c in range(NCH):
                        if c % 4 == 0:
                            ensure_proj(1, c // 4)
                        scores_unit(h, c, g)
                        if h == 0 and g == 1:
                            v_proj(c)
                        if started[0] and c % 4 == 3 and chains:
                            ctx_chain(*chains.pop(0))
                    chains.extend((h, qb) for qb in range(4 * g, 4 * g + 4))
                    if h == 0 and g == 1:
                        started[0] = True  # first chains drain during h0 g2
                if h == H_LOC - 1:
                    flush_slot()
                    while chains:
                        ctx_chain(*chains.pop(0))

    nc.compile()
    return nc


def build_kernel2(with_gamma):
    nc = bacc.Bacc("TRN2", target_bir_lowering=False, debug=False)

    R = 512  # rows per core
    ctxT = nc.dram_tensor("ctxT", [D, R], BF16, kind="ExternalInput")
    woT = nc.dram_tensor("woT", [D, D], BF16, kind="ExternalInput")
    xres = nc.dram_tensor("xres", [R, D], BF16, kind="ExternalInput")
    if with_gamma:
        gamma = nc.dram_tensor("gamma", [1, D], F32, kind="ExternalInput")
        beta = nc.dram_tensor("beta", [1, D], F32, kind="ExternalInput")
    out = nc.dram_tensor("out", [R, D], F32, kind="ExternalOutput")

    with tile.TileContext(nc) as tc:
        with (
            tc.tile_pool(name="wo", bufs=1) as wop,
            tc.tile_pool(name="cx", bufs=1) as cxp,
            tc.tile_pool(name="sm", bufs=1) as smp,
            tc.tile_pool(name="wk", bufs=3) as wkp,
            tc.tile_pool(name="ps", bufs=2, space="PSUM") as psp,
        ):
            wo_t = wop.tile([128, NIC, D], BF16)
            ctx_t = cxp.tile([128, NIC, R], BF16)
            xq_sb = cxp.tile([128, 4, D], BF16)
            # interleave ctx/wo chunk DMAs (2 ic blocks per DMA) so the
            # accumulation chain starts early and never starves; first xres
            # block early for the sc0 residual add.
            for cc in range(4):
                nc.sync.dma_start(
                    out=ctx_t[:, 2 * cc : 2 * cc + 2, :],
                    in_=_dram3(ctxT, 128, 2, R, off0=256 * cc * R, row_len=R),
                )
                nc.sync.dma_start(
                    out=wo_t[:, 2 * cc : 2 * cc + 2, :],
                    in_=_dram3(woT, 128, 2, D, off0=256 * cc * D, row_len=D),
                )
                if cc == 0:
                    nc.sync.dma_start(
                        out=xq_sb[:, 0, :], in_=_dram3(xres, 128, 1, D)
                    )
            nc.sync.dma_start(
                out=xq_sb[:, 1:4, :], in_=_dram3(xres, 128, 3, D, off0=128 * D)
            )

            if with_gamma:
                gb = smp.tile([128, D], F32)
                bb = smp.tile([128, D], F32)
                g_ap = gamma.ap()
                b_ap = beta.ap()
                nc.sync.dma_start(
                    out=gb[:], in_=bass.AP(tensor=g_ap.tensor, offset=g_ap.offset,
                                           ap=[[0, 128], [1, D]])
                )
                nc.sync.dma_start(
                    out=bb[:], in_=bass.AP(tensor=b_ap.tensor, offset=b_ap.offset,
                                           ap=[[0, 128], [1, D]])
                )
            eps_t = smp.tile([128, 1], F32)
            nc.vector.memset(eps_t[:], EPS)

            # out-projection + residual + LayerNorm, 128 rows at a time.
            # j-outer matmul order so residual-add + bn_stats of column half
            # j=0 overlap the j=1 matmuls.
            for sc in range(4):
                po = psp.tile([128, D], F32, tag="po")
                x_sb = wkp.tile([128, D], F32, tag="x")
                stats = wkp.tile([128, 2, 6], F32, tag="bn")
                for j in range(2):
                    for ic in range(NIC):
                        nc.tensor.matmul(
                            po[:, 512 * j : 512 * j + 512],
                            ctx_t[:, ic, 128 * sc : 128 * sc + 128],
                            wo_t[:, ic, 512 * j : 512 * j + 512],
                            start=(ic == 0),
                            stop=(ic == NIC - 1),
                        )
                    nc.vector.tensor_add(
                        out=x_sb[:, 512 * j : 512 * j + 512],
                        in0=po[:, 512 * j : 512 * j + 512],
                        in1=xq_sb[:, sc, 512 * j : 512 * j + 512],
                    )
                    nc.vector.bn_stats(
                        out=stats[:, j, :], in_=x_sb[:, 512 * j : 512 * j + 512]
                    )
                mv = wkp.tile([128, 2], F32, tag="mv")
                nc.vector.bn_aggr(out=mv[:], in_=stats[:])
                std = wkp.tile([128, 1], F32, tag="std")
                nc.scalar.activation(
                    out=std[:], in_=mv[:, 1:2],
                    func=mybir.ActivationFunctionType.Sqrt,
                    bias=eps_t[:], scale=1.0,
                )
                rstd = wkp.tile([128, 1], F32, tag="rstd")
                nc.vector.reciprocal(out=rstd[:], in_=std[:])
                if with_gamma:
                    xn = wkp.tile([128, D], F32, tag="xn")
                    nc.vector.tensor_scalar(
                        out=xn[:], in0=x_sb[:],
                        scalar1=mv[:, 0:1], scalar2=rstd[:],
                        op0=mybir.AluOpType.subtract, op1=mybir.AluOpType.mult,
                    )
                    xg = wkp.tile([128, D], F32, tag="xg")
                    nc.vector.tensor_mul(out=xg[:], in0=xn[:], in1=gb[:])
                    xb = wkp.tile([128, D], F32, tag="xb")
                    nc.vector.tensor_add(out=xb[:], in0=xg[:], in1=bb[:])
                    nc.sync.dma_start(
                        out=out[128 * sc : 128 * sc + 128, :], in_=xb[:]
                    )
                else:
                    xn = wkp.tile([128, D], F32, tag="xn")
                    nc.vector.tensor_scalar(
                        out=xn[:], in0=x_sb[:],
                        scalar1=mv[:, 0:1], scalar2=rstd[:],
                        op0=mybir.AluOpType.subtract, op1=mybir.AluOpType.mult,
                    )
                    nc.sync.dma_start(
                        out=out[128 * sc : 128 * sc + 128, :], in_=xn[:]
                    )

    nc.compile()
    return nc


def _get(name):
    if name not in _cache:
        if name == "k1":
            _cache[name] = build_kernel1()
        elif name == "k2":
            _cache[name] = build_kernel2(False)
        else:
            _cache[name] = build_kernel2(True)
    return _cache[name]


IDENT_HOST = np.eye(128, dtype=NPBF16)


def kernel(query, key, value, w_q, w_k, w_v, w_o, ln_gamma, ln_beta):
    query = np.asarray(query, np.float32)
    key = np.asarray(key, np.float32)
    value = np.asarray(value, np.float32)
    w_q = np.asarray(w_q, np.float32)
    w_k = np.asarray(w_k, np.float32)
    w_v = np.asarray(w_v, np.float32)
    w_o = np.asarray(w_o, np.float32)
    ln_gamma = np.asarray(ln_gamma, np.float32)
    ln_beta = np.asarray(ln_beta, np.float32)

    nc1 = _get("k1")
    plain_ln = bool(np.all(ln_gamma == 1.0) and np.all(ln_beta == 0.0))
    nc2 = _get("k2" if plain_ln else "k2g")

    xqT = [np.ascontiguousarray(query[b].T).astype(NPFP8) for b in range(B)]
    xkT = [np.ascontiguousarray(key[b].T).astype(NPFP8) for b in range(B)]
    xvT = [np.ascontiguousarray(value[b].T).astype(NPFP8) for b in range(B)]
    # packed per-head-group [wq | wk | wv] columns, fp8. q/k columns are
    # permuted to the DoubleRow scores layout: [dk-half, head, dk%32].
    perm = np.array(
        [64 * h + 32 * half + d for half in (0, 1) for h in range(4)
         for d in range(32)]
    )
    wq4 = np.asarray(w_q.T, np.float32).reshape(D, 4, 256)[:, :, perm]
    wk4 = np.asarray(w_k.T, np.float32).reshape(D, 4, 256)[:, :, perm]
    wv4 = np.asarray(w_v.T, np.float32).reshape(D, 4, 256)
    wqkvT = np.concatenate([wq4, wk4, wv4], axis=2).astype(NPFP8)  # [D, hg, 768]

    in_maps1 = []
    for c in range(N_CORES):
        b, hg = c // 4, c % 4
        in_maps1.append({
            "xq": xqT[b], "xk": xkT[b], "xv": xvT[b],
            "wqkv": np.ascontiguousarray(wqkvT[:, hg, :]),
            "ident": IDENT_HOST,
        })
    res1 = run_bass_kernel_spmd(nc1, in_maps1, core_ids=list(range(N_CORES)))

    ctxT_full = np.empty((D, B * S), NPBF16)
    for c in range(N_CORES):
        b, hg = c // 4, c % 4
        ctxT_full[256 * hg : 256 * hg + 256, S * b : S * b + S] = res1.results[c]["ctxT"]

    woT = np.ascontiguousarray(w_o.T).astype(NPBF16)
    q_flat = query.reshape(B * S, D)

    in_maps2 = []
    for c in range(N_CORES):
        r0 = 512 * c
        m = {
            "ctxT": np.ascontiguousarray(ctxT_full[:, r0 : r0 + 512]),
            "woT": woT,
            "xres": np.ascontiguousarray(q_flat[r0 : r0 + 512, :]).astype(NPBF16),
        }
        if not plain_ln:
            m["gamma"] = ln_gamma.reshape(1, D)
            m["beta"] = ln_beta.reshape(1, D)
        in_maps2.append(m)
    res2 = run_bass_kernel_spmd(nc2, in_maps2, core_ids=list(range(N_CORES)))

    out = np.concatenate([res2.results[c]["out"] for c in range(N_CORES)], axis=0)
    return out.reshape(B, S, D)
